# revision 1
# baseline (speedup 1.0000x reference)
"""DTCRF loss (nn_DTCRF_13091060318392) — Trainium2 Bass kernel, 8 NeuronCores.

Self-contained: takes FULL inputs (B=512, S=2048, N=49), shards the batch over
8 cores (64 rows each), runs the CRF forward recurrence + emission gather on
device, and assembles the scalar loss on host.

Device math per core (tag dim on partitions, batch on free axis):
  z_t = (z_{t-1} @ E) * exp(emit_t - MU),  E = exp(T)  [augmented ones column
  captures per-column sums for periodic rescaling; rank-1 PE matmul broadcasts
  the reciprocal; logs of the scale factors accumulate on device]
  den_b = m_b + (S-1)*MU + sum(ln s_k)    [last s_k = final column sum]
  emit-score gather: one-hot matmuls accumulated in PSUM, diagonal extracted.
Host: transition-score sum over tags, final reduction.
"""

import sys
import types
from contextlib import ExitStack

import numpy as np

# ---------------------------------------------------------------------------
# environment shims (NTFF profile hook absent in this image; walrus here
# supports at most one sync wait per instruction)
# ---------------------------------------------------------------------------


def _apply_ntff_shim():
    if "antenv.axon_hooks" not in sys.modules:
        mod = types.ModuleType("antenv.axon_hooks")
        mod._hook = None
        mod.set_axon_ntff_profile_hook = lambda h: setattr(mod, "_hook", h)
        mod.get_axon_ntff_profile_hook = lambda: mod._hook
        sys.modules["antenv.axon_hooks"] = mod
        try:
            import antenv

            antenv.axon_hooks = mod
        except ImportError:
            pass
    try:
        from antenv.axon_hooks import (
            get_axon_ntff_profile_hook,
            set_axon_ntff_profile_hook,
        )

        if get_axon_ntff_profile_hook() is None:
            from trn_agent_boot.trn_boot import _ntff_profile_via_ctypes

            set_axon_ntff_profile_hook(
                _ntff_profile_via_ctypes("/opt/axon/libaxon_pjrt.so")
            )
    except Exception:
        pass
    try:
        import concourse.bass_utils as bu

        bu.upload_artifacts = lambda tmpdir: f"file://{tmpdir}"
    except Exception:
        pass


def _split_multiwaits(nc):
    import bass_rust
    from concourse import mybir

    for bassbb in nc.bb_map.values():
        bb = bassbb.bb
        new = []
        changed = False
        for inst in bb.instructions:
            si = inst.sync_info
            waits = list(si.on_wait) if si and si.on_wait else []
            if len(waits) > 1:
                changed = True
                for k, w in enumerate(waits[:-1]):
                    nop = mybir.InstNoOp(name=f"{inst.name}_wsplit{k}", ins=[], outs=[])
                    nop.engine = inst.engine
                    nop.sync_info = bass_rust.SyncInfo(on_wait=[w], on_update=[])
                    try:
                        nc.register_instruction(nop)
                    except Exception:
                        pass
                    new.append(nop)
                si.on_wait = [waits[-1]]
                inst.sync_info = si
            new.append(inst)
        if changed:
            bb.instructions = new


# ---------------------------------------------------------------------------
# constants
# ---------------------------------------------------------------------------

N = 49
NP = 65  # stationary padded so the colsum row lands at partition 64
CS = 64  # colsum row index
BPC = 64  # batch rows per core
NCORES = 8
MU = 4.0
RK = 8
LAG = 4
LNB = 8

_NC_CACHE = {}


def _build_nc(S, KT=64):
    import concourse.bass as bass
    import concourse.tile as tile
    from concourse import mybir

    F32 = mybir.dt.float32
    BF16 = mybir.dt.bfloat16

    assert S % KT == 0
    n_slots = (S - 1) // RK + 1
    assert n_slots % LNB == 0, (S, n_slots)

    nc = bass.Bass()
    lt_d = nc.dram_tensor("lt", [N, S, BPC], BF16, kind="ExternalInput")
    oh_d = nc.dram_tensor("oh", [N, S, BPC], BF16, kind="ExternalInput")
    eh_d = nc.dram_tensor("eh", [N, NP], F32, kind="ExternalInput")
    ones_d = nc.dram_tensor("ones49", [NP, N], F32, kind="ExternalInput")
    i64_d = nc.dram_tensor("i64", [BPC, BPC], F32, kind="ExternalInput")
    z0_d = nc.dram_tensor("z0", [N, BPC], F32, kind="ExternalInput")
    c_out = nc.dram_tensor("c_out", [1, BPC], F32, kind="ExternalOutput")
    esc_out = nc.dram_tensor("esc_out", [BPC, 1], F32, kind="ExternalOutput")

    with tile.TileContext(nc) as tc, ExitStack() as ctx:
        singles = ctx.enter_context(tc.tile_pool(name="singles", bufs=1))
        ltp = ctx.enter_context(tc.tile_pool(name="ltp", bufs=2))
        ohp = ctx.enter_context(tc.tile_pool(name="ohp", bufs=2))
        eep = ctx.enter_context(tc.tile_pool(name="eep", bufs=2))
        zp = ctx.enter_context(tc.tile_pool(name="zp", bufs=3))
        rp = ctx.enter_context(tc.tile_pool(name="rp", bufs=2))
        smalls = ctx.enter_context(tc.tile_pool(name="smalls", bufs=2))
        up = ctx.enter_context(tc.tile_pool(name="up", bufs=3, space="PSUM"))
        gp = ctx.enter_context(tc.tile_pool(name="gp", bufs=1, space="PSUM"))
        Rp = ctx.enter_context(tc.tile_pool(name="Rp", bufs=2, space="PSUM"))

        eh_s = singles.tile([N, NP], F32)
        nc.sync.dma_start(out=eh_s, in_=eh_d[:])
        ones_s = singles.tile([NP, N], F32)
        nc.sync.dma_start(out=ones_s, in_=ones_d[:])
        i64_s = singles.tile([BPC, BPC], F32)
        nc.sync.dma_start(out=i64_s, in_=i64_d[:])

        mubias = singles.tile([N, 1], F32)
        nc.vector.memset(mubias, -MU)

        z_cur = zp.tile([N, BPC], F32, tag="z")
        nc.sync.dma_start(out=z_cur, in_=z0_d[:])
        s_hist = singles.tile([NP, LNB * BPC], F32)
        c_acc = singles.tile([NP, BPC], F32)
        nc.vector.memset(c_acc[CS : CS + 1, :], 0.0)
        psum_g = gp.tile([BPC, BPC], F32)

        state = {"pending": None, "slot": 0}

        def capture_and_rescale(u, t, final=False):
            j = state["slot"] % LNB
            nc.scalar.copy(
                out=s_hist[CS : CS + 1, j * BPC : (j + 1) * BPC],
                in_=u[CS : CS + 1, :],
            )
            if not final:
                r_t = rp.tile([NP, BPC], F32, tag="r")
                nc.vector.reciprocal(out=r_t[CS : CS + 1, :], in_=u[CS : CS + 1, :])
                R = Rp.tile([N, BPC], F32, tag="R")
                nc.tensor.matmul(
                    R,
                    ones_s[CS : CS + 1, :],
                    r_t[CS : CS + 1, :],
                    start=True,
                    stop=True,
                )
                state["pending"] = (R, t + LAG)
            state["slot"] += 1
            if state["slot"] % LNB == 0:
                lh = smalls.tile([NP, LNB * BPC], F32, tag="lh")
                nc.scalar.activation(
                    out=lh[CS : CS + 1, :],
                    in_=s_hist[CS : CS + 1, :],
                    func=mybir.ActivationFunctionType.Ln,
                )
                lrow = lh[CS : CS + 1, :]
                view = bass.AP(
                    tensor=lrow.tensor,
                    offset=lrow.offset,
                    ap=[[lrow.ap[0][0], 1], [1, BPC], [BPC, LNB]],
                )
                csub = smalls.tile([NP, BPC], F32, tag="csub")
                nc.vector.tensor_reduce(
                    out=csub[CS : CS + 1, :],
                    in_=view,
                    axis=mybir.AxisListType.X,
                    op=mybir.AluOpType.add,
                )
                nc.vector.tensor_add(
                    c_acc[CS : CS + 1, :],
                    c_acc[CS : CS + 1, :],
                    csub[CS : CS + 1, :],
                )

        for ci in range(S // KT):
            t0 = ci * KT
            lt_c = ltp.tile([N, KT, BPC], BF16, tag="lt")
            nc.sync.dma_start(out=lt_c, in_=lt_d[:, t0 : t0 + KT, :])
            oh_c = ohp.tile([N, KT, BPC], BF16, tag="oh")
            nc.sync.dma_start(out=oh_c, in_=oh_d[:, t0 : t0 + KT, :])
            ee_c = eep.tile([N, KT, BPC], BF16, tag="ee")
            nc.scalar.activation(
                out=ee_c,
                in_=lt_c,
                func=mybir.ActivationFunctionType.Exp,
                bias=mubias[:, :],
                scale=1.0,
            )
            for k in range(KT):
                t = t0 + k
                nc.tensor.matmul(
                    psum_g,
                    oh_c[:, k, :],
                    lt_c[:, k, :],
                    start=(t == 0),
                    stop=(t == S - 1),
                )
                if t == 0:
                    continue
                u = up.tile([NP, BPC], F32, tag="u")
                nc.tensor.matmul(u, eh_s, z_cur, start=True, stop=True)
                z_nxt = zp.tile([N, BPC], F32, tag="z")
                nc.vector.tensor_mul(z_nxt, u[0:N, :], ee_c[:, k, :])
                if t % RK == 0 and t + LAG <= S - 1:
                    capture_and_rescale(u, t)
                if state["pending"] is not None and t == state["pending"][1]:
                    nc.vector.tensor_mul(z_nxt, z_nxt, state["pending"][0])
                    state["pending"] = None
                z_cur = z_nxt

        u = up.tile([NP, BPC], F32, tag="u")
        nc.tensor.matmul(u, eh_s, z_cur, start=True, stop=True)
        capture_and_rescale(u, S - 1, final=True)

        scratch = smalls.tile([BPC, BPC], F32, tag="scratch")
        esc_t = smalls.tile([BPC, 1], F32, tag="esc")
        nc.vector.tensor_mul(scratch, psum_g, i64_s)
        nc.vector.tensor_reduce(
            out=esc_t,
            in_=scratch,
            axis=mybir.AxisListType.X,
            op=mybir.AluOpType.add,
        )

        nc.sync.dma_start(out=c_out[:], in_=c_acc[CS : CS + 1, :])
        nc.sync.dma_start(out=esc_out[:], in_=esc_t)

    _split_multiwaits(nc)
    return nc


def _build_transitions_np(p_in, p_cross, p_out, p_to_out, p_from_out):
    E, M = 12, 4
    eye = np.eye(E, dtype=bool)
    blocks = np.where(eye[:, :, None, None], p_in, p_cross)
    inner = blocks.transpose(0, 2, 1, 3).reshape(E * M, E * M)
    T = np.zeros((N, N), dtype=np.float32)
    T[1:, 1:] = inner
    T[0, 0] = p_out[0]
    T[0, 1:] = np.tile(p_from_out, E)
    T[1:, 0] = np.tile(p_to_out, E)
    return T


def _ref_numpy_general(inputs, tags, mask, T):
    """Slow but general fallback (used only if mask is not all ones)."""
    B, S, _ = inputs.shape
    Tf = T.astype(np.float64)
    lg = inputs.astype(np.float64)
    alpha = lg[:, 0, :]
    for t in range(1, S):
        inner = alpha[:, :, None] + Tf[None, :, :] + lg[:, t, None, :]
        m = inner.max(axis=1, keepdims=True)
        new_alpha = np.log(np.exp(inner - m).sum(axis=1)) + m[:, 0, :]
        alpha = np.where((mask[:, t] > 0)[:, None], new_alpha, alpha)
    am = alpha.max(1)
    den = np.log(np.exp(alpha - am[:, None]).sum(1)) + am
    fm = mask.astype(np.float64)
    tg = tags.astype(np.int64)
    trans = (Tf[tg[:, :-1], tg[:, 1:]] * fm[:, 1:]).sum(1)
    emit = (
        np.take_along_axis(lg[:, :-1, :], tg[:, :-1, None], axis=2)[:, :, 0]
        * fm[:, :-1]
    ).sum(1)
    last_idx = mask.sum(1).astype(np.int64) - 1
    last_tags = np.take_along_axis(tg, last_idx[:, None], axis=1)[:, 0]
    last_emit = lg[np.arange(B), -1, last_tags]
    num = trans + emit + last_emit * fm[:, -1]
    return np.float32(np.sum(num - den))


def kernel(inputs, tags, mask, p_in, p_cross, p_out, p_to_out, p_from_out):
    import ml_dtypes

    B, S, Nn = inputs.shape
    T = _build_transitions_np(
        np.asarray(p_in, np.float32),
        np.asarray(p_cross, np.float32),
        np.asarray(p_out, np.float32),
        np.asarray(p_to_out, np.float32),
        np.asarray(p_from_out, np.float32),
    )

    if not np.all(np.asarray(mask) == 1):
        return _ref_numpy_general(np.asarray(inputs), np.asarray(tags), np.asarray(mask), T)

    _apply_ntff_shim()
    from concourse.bass_utils import run_bass_kernel_spmd

    key = (S,)
    if key not in _NC_CACHE:
        _NC_CACHE[key] = _build_nc(S)
    nc = _NC_CACHE[key]

    E = np.exp(T.astype(np.float32))
    eh = np.zeros((N, NP), np.float32)
    eh[:, :N] = E
    eh[:, CS] = 1.0
    ones49 = np.zeros((NP, N), np.float32)
    ones49[CS, :] = 1.0
    i64 = np.eye(BPC, dtype=np.float32)

    tags32 = np.asarray(tags).astype(np.int32)
    ar = np.arange(N, dtype=np.int32)
    inputs = np.asarray(inputs, dtype=np.float32)

    in_maps = []
    m_all = np.empty(B, np.float32)
    for c in range(NCORES):
        b0 = c * BPC
        blk = inputs[b0 : b0 + BPC]
        lt = np.ascontiguousarray(blk.transpose(2, 1, 0)).astype(ml_dtypes.bfloat16)
        tg = tags32[b0 : b0 + BPC]
        oh = (ar[:, None, None] == tg.T[None, :, :]).astype(ml_dtypes.bfloat16)
        a0 = blk[:, 0, :]
        m = a0.max(axis=1)
        z0 = np.ascontiguousarray(np.exp(a0 - m[:, None]).T.astype(np.float32))
        m_all[b0 : b0 + BPC] = m
        in_maps.append(
            {"lt": lt, "oh": oh, "eh": eh, "ones49": ones49, "i64": i64, "z0": z0}
        )

    trans = T[tags32[:, :-1], tags32[:, 1:]].astype(np.float64).sum(axis=1)

    res = run_bass_kernel_spmd(nc, in_maps, core_ids=list(range(NCORES)))

    total = 0.0
    for c in range(NCORES):
        b0 = c * BPC
        c_b = res.results[c]["c_out"].reshape(BPC).astype(np.float64)
        esc = res.results[c]["esc_out"].reshape(BPC).astype(np.float64)
        m = m_all[b0 : b0 + BPC].astype(np.float64)
        den = m + (S - 1) * MU + c_b
        num = esc + trans[b0 : b0 + BPC]
        total += float(np.sum(num - den))
    return np.float32(total)

